# revision 14
# baseline (speedup 1.0000x reference)
"""Bahdanau additive attention on Trainium2 (Bass/Tile), SPMD over 8 NeuronCores.

Problem: attn_out[b,t,:] = softmax_s(v . tanh(enc_f[b,s,:] + qry_f[b,t,:])) @ enc[b]
  with enc_f = enc @ W_h^T, qry_f = q @ W_s^T, masked to s < src_lengths[b].

Sharding: parallel over tgt_len T — core i handles query rows [i*32,(i+1)*32)
for ALL batches; weights and encoder outputs replicated. Every core does the
same amount of work so load is balanced despite variable src_lengths.

Layout: hidden dim h on SBUF partitions (4 chunks of 128). Per query row t,
tanh(enc_fT[h,s] + qry_fT[h,t]) is a per-partition broadcast add (DVE
tensor_scalar 4x bf16, a slice offloaded to GPSIMD), tanh runs as one big ACT
instruction per (batch, chunk) over all 32 rows (ACT is the critical engine:
~1 elem/cycle/lane), and the v-reduction over h is a col-tiled M=32 matmul:
groups of r=2/4 query rows go to distinct 32-partition col-groups via
tile_position so up to 4 matmuls execute concurrently in the PE array.

Scores land in PSUM at partitions {0,32,64,96}; a PSUM->SBUF stage copy plus
one gather DMA per batch produces the [32, S] softmax layout. Scores are
bounded (|score| <= sum|v| ~ 18) so exp runs without max subtraction; the
activation's accum_out produces the row sums for free.

src_lengths are read on the host at trace time: loop extents are specialized
to L_b (padded to a multiple of 4); masked positions are never computed.
"""

import math
import os

import numpy as np

NCORES = 8
P = 128


def _build_program(B, T_core, S, H, L, Lh, reps=1):
    import concourse.bass as bass  # noqa: F401
    import concourse.mybir as mybir
    import concourse.tile as tile
    from concourse import bacc
    from concourse.masks import make_identity

    f32 = mybir.dt.float32
    bf16 = mybir.dt.bfloat16
    AF = mybir.ActivationFunctionType

    HC = H // P  # h chunks (4)

    # Bacc (not raw Bass): its compile() legalizes sync waits (matmuls can
    # carry at most one wait in hardware; extra waits move to ldweights /
    # event semaphores).
    nc = bacc.Bacc("TRN2", target_bir_lowering=False, debug=False)

    enc_d = nc.declare_dram_parameter("enc", [B, S, H], f32, isOutput=False)
    q_d = nc.declare_dram_parameter("q", [B, T_core, H], f32, isOutput=False)
    wh_d = nc.declare_dram_parameter("wh", [H, H], f32, isOutput=False)
    ws_d = nc.declare_dram_parameter("ws", [H, H], f32, isOutput=False)
    v_d = nc.declare_dram_parameter("v", [H], f32, isOutput=False)
    out_d = nc.declare_dram_parameter("out", [B, T_core, H], f32, isOutput=True)

    with tile.TileContext(nc) as tc:
        with (
            tc.tile_pool(name="const", bufs=1) as constp,
            tc.tile_pool(name="sb", bufs=2) as sb,
            tc.tile_pool(name="work", bufs=2) as workp,
            tc.tile_pool(name="ps", bufs=2, space="PSUM") as psp,
            tc.tile_pool(name="ps_sc", bufs=1, space="PSUM") as pssc,
        ):
            ident_f = constp.tile([P, P], f32)
            make_identity(nc, ident_f)
            ident_b = constp.tile([P, P], bf16)
            make_identity(nc, ident_b)

            # v -> [128, HC] f32 -> bf16 (column c = chunk c of v)
            v_f = constp.tile([P, HC], f32)
            nc.sync.dma_start(v_f, v_d.rearrange("(c p) -> p c", p=P))
            # v32 block c = [v chunk c, 0 x 31]: M=32 score matmuls write all
            # 32 partitions of a col-group so downstream PSUM reads are fully
            # initialized; matmul cost depends only on N.
            v32 = constp.tile([P, HC * 32], bf16)
            nc.vector.memset(v32, 0.0)
            for c in range(HC):
                nc.vector.tensor_copy(v32[:, c * 32 : c * 32 + 1], v_f[:, c : c + 1])

            # W_h^T in bf16, W_s^T in f32.
            # whT block k (cols [k*H,(k+1)*H)) = W_h^T[h' in chunk k, :]
            whT = constp.tile([P, HC * H], bf16)
            wsT = constp.tile([P, HC * H], f32)
            whnb = []
            wsn = []
            for c in range(HC):
                wn = sb.tile([P, H], f32, name=f"whn{c}", tag="wn", bufs=4)
                nc.sync.dma_start(wn, wh_d[c * P : (c + 1) * P, :])
                wnb = sb.tile([P, H], bf16, name=f"whnb{c}", tag="wnb", bufs=4)
                nc.vector.tensor_copy(wnb, wn)
                whnb.append(wnb)
                wsn_c = sb.tile([P, H], f32, name=f"wsn{c}", tag="wsn", bufs=4)
                nc.sync.dma_start(wsn_c, ws_d[c * P : (c + 1) * P, :])
                wsn.append(wsn_c)
            for k in range(HC):
                pst = psp.tile([P, HC * P], bf16, name=f"whT_ps{k}", tag="mmC", bufs=2)
                for c in range(HC):
                    nc.tensor.transpose(
                        pst[:, c * P : (c + 1) * P],
                        whnb[c][:, k * P : (k + 1) * P],
                        ident_b,
                    )
                nc.vector.tensor_copy(whT[:, k * H : (k + 1) * H], pst)
            for k in range(HC):
                psf = psp.tile([P, HC * P], f32, name=f"wsT_ps{k}", tag="mmC", bufs=2)
                for c in range(HC):
                    nc.tensor.transpose(
                        psf[:, c * P : (c + 1) * P],
                        wsn[c][:, k * P : (k + 1) * P],
                        ident_f,
                    )
                nc.vector.tensor_copy(wsT[:, k * H : (k + 1) * H], psf)

            def load(b):
                """DMA the encoder rows (valid range only) + query slice."""
                Lhb = Lh[b]
                nk = (Lhb + P - 1) // P
                enc_nat = []
                for k2 in range(nk):
                    r2 = min(P, Lhb - k2 * P)
                    en = sb.tile([P, H], f32, name=f"enc{b}_{k2}", tag=f"enc{k2}", bufs=3)
                    nc.sync.dma_start(
                        en[:r2, :], enc_d[b, k2 * P : k2 * P + r2, :]
                    )
                    enc_nat.append((en, r2))
                qn = sb.tile([T_core, H], f32, name=f"qn{b}", tag="qn", bufs=3)
                nc.sync.dma_start(qn, q_d[b])
                return enc_nat, qn

            def phase_a(b, enc_nat, qn):
                """encT, enc_fT (bf16) and qry_fT (f32) for batch b."""
                Lhb = Lh[b]
                nk = (Lhb + P - 1) // P
                # encT (bf16): block k = enc^T[h' in chunk k, s]
                encT = sb.tile([P, HC * S], bf16, name=f"encT{b}", tag="encT", bufs=3)
                encT_v = encT.rearrange("p (k s) -> p k s", k=HC)
                for k2 in range(nk):
                    en, r2 = enc_nat[k2]
                    ps_t = psp.tile(
                        [P, HC * P], f32, name=f"encT_ps{b}_{k2}", tag="mmA", bufs=1
                    )
                    for k in range(HC):
                        nc.tensor.transpose(
                            ps_t[:, k * P : k * P + r2],
                            en[:r2, k * P : (k + 1) * P],
                            ident_f[:r2, :r2],
                        )
                    nc.vector.tensor_copy(
                        encT_v[:, :, k2 * P : k2 * P + r2],
                        ps_t.rearrange("p (k s) -> p k s", k=HC)[:, :, :r2],
                    )
                # enc_fT (bf16): block c = (W_h @ enc^T)[h in chunk c, s]
                ps_e = psp.tile([P, HC * S], f32, name=f"encf_ps{b}", tag="mmA", bufs=1)
                for c in range(HC):
                    for k in range(HC):
                        nc.tensor.matmul(
                            ps_e[:, c * S : c * S + Lhb],
                            whT[:, k * H + c * P : k * H + (c + 1) * P],
                            encT_v[:, k, :Lhb],
                            start=(k == 0),
                            stop=(k == HC - 1),
                        )
                encfT = sb.tile([P, HC * S], bf16, name=f"encfT{b}", tag="encfT", bufs=3)
                nc.vector.tensor_copy(
                    encfT.rearrange("p (c s) -> p c s", c=HC)[:, :, :Lhb],
                    ps_e.rearrange("p (c s) -> p c s", c=HC)[:, :, :Lhb],
                )
                # qry_fT (f32): block c cols = (W_s @ q^T)[h in chunk c, t]
                ps_q = psp.tile([P, HC * T_core], f32, name=f"qT_ps{b}", tag="mmA", bufs=1)
                for k in range(HC):
                    nc.tensor.transpose(
                        ps_q[:, k * T_core : (k + 1) * T_core],
                        qn[:, k * P : (k + 1) * P],
                        ident_f[:T_core, :T_core],
                    )
                qT = sb.tile([P, HC * T_core], f32, name=f"qT{b}", tag="qT", bufs=3)
                nc.vector.tensor_copy(qT, ps_q)
                ps_qf = psp.tile([P, HC * T_core], f32, name=f"qf_ps{b}", tag="mmA", bufs=1)
                for c in range(HC):
                    for k in range(HC):
                        nc.tensor.matmul(
                            ps_qf[:, c * T_core : (c + 1) * T_core],
                            wsT[:, k * H + c * P : k * H + (c + 1) * P],
                            qT[:, k * T_core : (k + 1) * T_core],
                            start=(k == 0),
                            stop=(k == HC - 1),
                        )
                qfT = sb.tile([P, HC * T_core], f32, name=f"qfT{b}", tag="qfT", bufs=3)
                nc.vector.tensor_copy(qfT, ps_qf)
                return encfT, qfT

            def phase_bc(b, enc_nat, encfT, qfT):
                Lhb = Lh[b]
                nk = (Lhb + P - 1) // P
                # rows per score matmul: keep r*Lhb <= 512 (one PSUM bank run)
                r = 4 if Lhb <= 128 else 2
                G = 16 // r  # groups per 16-row sweep (4 or 8)
                nslot = G // 4  # bank slots per sweep (1 or 2)

                # ---- broadcast adds + tanh, all 32 rows per (b, chunk) ----
                tanh_c = []
                for c in range(HC):
                    sum_t = workp.tile(
                        [P, T_core * Lhb], bf16, name=f"sum{b}_{c}",
                        tag="sum", bufs=2,
                    )
                    for t in range(T_core):
                        # offload 1 in 4 adds to otherwise-idle GPSIMD
                        eng = nc.gpsimd if t % 4 == 3 else nc.vector
                        eng.tensor_scalar_add(
                            sum_t[:, t * Lhb : (t + 1) * Lhb],
                            encfT[:, c * S : c * S + Lhb],
                            qfT[:, c * T_core + t : c * T_core + t + 1],
                        )
                    tanh_t = workp.tile(
                        [P, T_core * Lhb], bf16, name=f"tanh{b}_{c}",
                        tag="tanh", bufs=4,
                    )
                    nc.scalar.activation(tanh_t, sum_t, AF.Tanh)
                    tanh_c.append(tanh_t)

                # ---- scores: col-tiled M=32 matmuls, r rows per matmul ----
                # group g of sweep: rows r*g..r*g+r-1 -> col-group (g%4),
                # bank-slot (g//4); scores for row r*g+j at partition
                # 32*(g%4), cols 512*(g//4) + j*Lhb.
                # chunk-outer order so tanh tile c is fully consumed (and its
                # slot freed) before tanh c+3 is needed; accumulation groups
                # interleave across PSUM regions, which has_written handles.
                ps_scs = [
                    pssc.tile(
                        [P, 2 * 512], f32, name=f"sc_ps{b}_{sweep}",
                        tag="scores", bufs=2,
                    )
                    for sweep in range(2)
                ]
                for c in range(HC):
                    for sweep in range(2):
                        for g in range(G):
                            cg, slot = g % 4, g // 4
                            t0 = sweep * 16 + r * g
                            nc.tensor.matmul(
                                ps_scs[sweep][
                                    32 * cg : 32 * cg + 32,
                                    512 * slot : 512 * slot + r * Lhb,
                                ],
                                v32[:, c * 32 : (c + 1) * 32],
                                tanh_c[c][:, t0 * Lhb : (t0 + r) * Lhb],
                                start=(c == 0),
                                stop=(c == HC - 1),
                                tile_position=(0, 32 * cg),
                                # group-tracking guard assumes 1 bank per
                                # partition; groups here are disjoint by
                                # (partition range, bank) so HW is safe
                                skip_group_check=True,
                            )
                scores_sw = []
                for sweep in range(2):
                    ps_sc = ps_scs[sweep]
                    # PSUM -> SBUF staging copy (partition-preserving)
                    stage = sb.tile(
                        [P, 1024], f32, name=f"stage{b}_{sweep}", tag="stage"
                    )
                    nc.vector.tensor_copy(
                        stage.rearrange("p (sl x) -> p sl x", sl=2)[
                            :, :nslot, : r * Lhb
                        ],
                        ps_sc.rearrange("p (sl x) -> p sl x", sl=2)[
                            :, :nslot, : r * Lhb
                        ],
                    )
                    # gather DMAs (one per bank slot): stage -> [16, S] layout
                    # row 4*r*sl + r*cg + j <- partition 32*cg, col 512*sl + j*Lhb
                    sc_w = sb.tile(
                        [16, S], f32, name=f"scores{b}_{sweep}", tag="scsb", bufs=4
                    )
                    for slot in range(nslot):
                        src = stage.rearrange(
                            "(cg p) (sl x) -> p sl cg x", cg=4, sl=2
                        )[0, slot, :, : r * Lhb].rearrange(
                            "cg (j s) -> cg j s", j=r
                        )
                        # plain row-slice dst so tile dep tracking sees the
                        # two slot writes as distinct regions
                        rows = 4 * r
                        dst = sc_w[slot * rows : (slot + 1) * rows, :Lhb]
                        nc.sync.dma_start(dst, src)
                    scores_sw.append(sc_w)
                return scores_sw

            def phase_c(b, enc_nat, scores_sw):
                # exact (unpadded) length here: the pad cols [L, Lh) hold
                # real scores that must NOT receive softmax weight
                Lb = L[b]
                nk = (Lb + P - 1) // P
                # ---- per-sweep softmax + attention ----
                # scores are bounded (|score| <= sum|v_h| ~ 18), so exp
                # without max subtraction is safe in f32.
                for sweep in range(2):
                    sc_w = scores_sw[sweep]
                    w_sw = sb.tile([16, S], f32, name=f"w{b}_{sweep}", tag="w")
                    sums = sb.tile([16, 1], f32, name=f"sums{b}_{sweep}", tag="sums")
                    nc.scalar.activation(
                        w_sw[:, :Lb], sc_w[:, :Lb], AF.Exp, accum_out=sums
                    )
                    recip = sb.tile([16, 1], f32, name=f"recip{b}_{sweep}", tag="recip")
                    nc.vector.reciprocal(recip, sums)

                    # attn_out = (w_raw @ enc) * recip
                    ps_w = psp.tile([P, 2 * 16], f32, name=f"wT_ps{b}_{sweep}", tag="mmC", bufs=2)
                    wT = sb.tile([P, 2 * 16], f32, name=f"wT{b}_{sweep}", tag="wT")
                    for k2 in range(nk):
                        r2 = min(P, Lb - k2 * P)
                        nc.tensor.transpose(
                            ps_w[:r2, k2 * 16 : (k2 + 1) * 16],
                            w_sw[:, k2 * P : k2 * P + r2],
                            ident_f[:16, :16],
                        )
                        nc.vector.tensor_copy(
                            wT[:r2, k2 * 16 : (k2 + 1) * 16],
                            ps_w[:r2, k2 * 16 : (k2 + 1) * 16],
                        )
                    ps_attn = psp.tile([16, H], f32, name=f"attn_ps{b}_{sweep}", tag="mmC", bufs=2)
                    for k2 in range(nk):
                        en, _ = enc_nat[k2]
                        r2 = min(P, Lb - k2 * P)
                        nc.tensor.matmul(
                            ps_attn,
                            wT[:r2, k2 * 16 : (k2 + 1) * 16],
                            en[:r2, :],
                            start=(k2 == 0),
                            stop=(k2 == nk - 1),
                        )
                    out_sb = sb.tile([16, H], f32, name=f"out{b}_{sweep}", tag="outsb")
                    nc.vector.tensor_scalar_mul(out_sb, ps_attn, recip)
                    nc.sync.dma_start(
                        out_d[b, sweep * 16 : (sweep + 1) * 16, :], out_sb
                    )

            def batch_loop():
                # two-deep software pipeline: phase A of b+1 is emitted ahead
                # of phase B of b, and phase C of b is emitted AFTER phase B
                # of b+1 — so exp(b) sits behind tanh(b+1) in the ACT FIFO
                # and never stalls the critical tanh stream.
                st = load(0)
                pa = phase_a(0, *st)
                sw = phase_bc(0, st[0], *pa)
                for b in range(B):
                    if b + 1 < B:
                        nxt_st = load(b + 1)
                        nxt_pa = phase_a(b + 1, *nxt_st)
                        nxt_sw = phase_bc(b + 1, nxt_st[0], *nxt_pa)
                    phase_c(b, st[0], sw)
                    if b + 1 < B:
                        st, pa, sw = nxt_st, nxt_pa, nxt_sw

            if reps > 1:
                # device-side repetition loop, used only for timing
                with tc.For_i(0, reps, 1):
                    batch_loop()
            else:
                batch_loop()

    nc.compile()
    return nc


LAST_EXEC_NS = None


def _get_program(key):
    # Build fresh every time: lowering through bass2jax mutates the nc
    # (partition-id preamble), so an nc must not be lowered twice.
    B, T_core, S, H, L, Lh = key
    return _build_program(B, T_core, S, H, list(L), list(Lh))


def kernel(query, encoder_outputs, src_lengths, W_h, W_s, v):
    global LAST_EXEC_NS
    from concourse.bass_utils import run_bass_kernel_spmd

    query = np.ascontiguousarray(np.asarray(query, dtype=np.float32))
    enc = np.ascontiguousarray(np.asarray(encoder_outputs, dtype=np.float32))
    W_h = np.ascontiguousarray(np.asarray(W_h, dtype=np.float32))
    W_s = np.ascontiguousarray(np.asarray(W_s, dtype=np.float32))
    v = np.ascontiguousarray(np.asarray(v, dtype=np.float32)).reshape(-1)
    L = [int(x) for x in np.asarray(src_lengths).reshape(-1)]

    B, T, H = query.shape
    S = enc.shape[1]
    T_core = T // NCORES
    Lh = [min(S, ((l + 3) // 4) * 4) for l in L]

    nc = _get_program((B, T_core, S, H, tuple(L), tuple(Lh)))

    in_maps = [
        {
            "enc": enc,
            "q": np.ascontiguousarray(query[:, i * T_core : (i + 1) * T_core, :]),
            "wh": W_h,
            "ws": W_s,
            "v": v,
        }
        for i in range(NCORES)
    ]
    res = run_bass_kernel_spmd(nc, in_maps, list(range(NCORES)))
    LAST_EXEC_NS = res.exec_time_ns
    out = np.concatenate([res.results[i]["out"] for i in range(NCORES)], axis=1)
    return out
